# revision 33
# baseline (speedup 1.0000x reference)
"""AttnBlock (GroupNorm -> qkv 1x1 -> softmax attention -> proj -> residual)
for x (2, 512, 64, 64) on 8 Trainium2 NeuronCores.

Sharding: core i handles batch i//4 and query-token block i%4 (1024 of 4096
spatial tokens). k/v are computed per-core over all 4096 tokens (no
collectives). Inputs are token-rolled per core so every core runs the same
SPMD graph with its own query block at token offset 0.

Following the original baseline's host-prep style (wo@wv product, bias and
residual folds, dtype casts), the cheap per-channel algebra is folded on the
host: GroupNorm reduces to hn = a*x + d (a,d per channel from exact f32
moments) and is absorbed into fp8 copies of the projection weights, with
per-tensor power-of-two prescales so fp8 sees a healthy range (undone by
the evacuation/activation scales on device). The ~86 GFLOP of projections
and attention all run on device.

The k projection is eliminated algebraically: S^T = K^T Q = x8^T (Wk_s Q),
so a small m8 = Wk^T q8 replaces the full 4096-token K — the S^T matmuls
use raw x8 chunks as stationary. exp(s/4 - 3) streams PSUM->SBUF into fp8
P^T (no max pass; the shift keeps fp8 in range and cancels in the softmax
ratio). The vw projection interleaves with the S^T/exp stream to fill the
PE while exp runs on ACT. A@V is all-fp8 DoubleRow with P^T chunks
stationary; the softmax denominator folds into AV via a ones-column in vw8
with each AV step bank-split (cols 0:258 -> bank0, 258:516 -> bank1), so
den[q] lands per-partition at pav[:,766] for free.
"""

import numpy as np

C = 512          # channels
N = 4096         # spatial tokens (64*64)
NB = 1024        # query tokens per core
G = 32           # groups
CT = 4           # channel tiles of 128
EPS = 1e-6
QKSCALE = float(C) ** -0.25  # split between q and the k-side fold
NCORES = 8
VW = 516         # vw8 inner: 512 ch + ones col + 3 zero pad
AVS = 258        # AV bank split point
EXPSHIFT = -3.0  # exp(s-3): keeps fp8 P below e4m3's 240 max (scores ~ +-7.5)
WQLAM = 16.0     # host prescale on folded Wq for fp8 range
WKLAM = 8.0      # host prescale on raw Wk for fp8 range
WVLAM = 8.0      # host prescale on folded WoWv for fp8 range
M8LAM = 4.0      # extra prescale kept inside m8, undone by exp input scale

_cache = {}


def _split_sync_waits(nc, maxw=1):
    """This walrus build encodes at most ~1 sync wait per instruction
    descriptor. Move excess sem waits onto same-engine nops inserted just
    before the instruction (in-order sequencers make this equivalent)."""
    from concourse import mybir

    n = 0
    for fn in nc.m.functions:
        for b in fn.blocks:
            out = []
            for ins in b.instructions:
                si = getattr(ins, "sync_info", None)
                if si is not None and si.on_wait and len(si.on_wait) > maxw:
                    waits = list(si.on_wait)
                    extra, keep = waits[:-maxw], waits[-maxw:]
                    for j in range(0, len(extra), maxw):
                        nop = mybir.InstNoOp(name=f"I-wsp{n}", ins=[], outs=[])
                        n += 1
                        nop.engine = ins.engine
                        nop.sync_info = mybir.SyncInfo(
                            on_wait=extra[j : j + maxw], on_update=[]
                        )
                        out.append(nop)
                    ins.sync_info = mybir.SyncInfo(
                        on_wait=keep, on_update=list(si.on_update)
                    )
                out.append(ins)
            b.instructions = out


def build(split_waits=True):
    import concourse.bass as bass
    import concourse.tile as tile
    from concourse import mybir

    f32 = mybir.dt.float32
    bf16 = mybir.dt.bfloat16
    fp8 = mybir.dt.float8e4
    ALU = mybir.AluOpType
    ACT = mybir.ActivationFunctionType
    DROW = mybir.MatmulPerfMode.DoubleRow

    nc = bass.Bass()
    X8 = nc.declare_dram_parameter("x_f8", [CT, 128, N], fp8, isOutput=False)
    WQ8 = nc.declare_dram_parameter("wq_s8", [C, C], fp8, isOutput=False)
    WKN = nc.declare_dram_parameter("wk_n8", [C, C], fp8, isOutput=False)
    WOV8 = nc.declare_dram_parameter("wov_s8", [C, C], fp8, isOutput=False)
    SML = nc.declare_dram_parameter("smalls", [128, CT, 2], f32, isOutput=False)
    XRT = nc.declare_dram_parameter("xres_t", [128, NB // 128, C], f32, isOutput=False)
    OUT = nc.declare_dram_parameter("out", [NB // 128, 128, C], f32, isOutput=True)

    w_re = {
        "q": WQ8.rearrange("(a p) o -> p a o", p=128),
        "kn": WKN.rearrange("(a p) o -> p a o", p=128),
        "ov": WOV8.rearrange("(a p) o -> p a o", p=128),
    }

    with tile.TileContext(nc) as tc:
        with (
            tc.tile_pool(name="persist", bufs=1) as persist,
        ):
            # persistent tensors
            xrt = persist.tile([128, NB // 128, C], f32)
            wq_s = persist.tile([128, CT, C], fp8)
            wkn8 = persist.tile([128, CT, C], fp8)
            wov_s = persist.tile([128, CT, C], fp8)
            q8 = persist.tile([128, CT, NB], fp8)
            m8 = persist.tile([128, CT, NB], fp8)
            vw8 = persist.tile([128, N // 128, VW], fp8)
            p8 = persist.tile([128, N // 128, NB], fp8)
            expshift = persist.tile([128, 1], f32)

            with (
                tc.tile_pool(name="xbp", bufs=1) as xbp,
                tc.tile_pool(name="statp", bufs=2) as statp,
                tc.tile_pool(name="ps_warm", bufs=1, space="PSUM") as ps_warm,
            ):
                # folded per-channel scalars (per core/batch, from host):
                # [:, :, 0] = q bias (after GN fold), [:, :, 1] = m8 fold
                sml_t = statp.tile([128, CT, 2], f32, tag="sml")
                nc.sync.dma_start(out=sml_t, in_=SML[:, :, :])
                bias_q = sml_t[:, :, 0:1]
                aqm_t = sml_t[:, :, 1:2]
                warm_sb = statp.tile([128, 512], bf16, tag="warm_sb")
                nc.vector.memset(warm_sb[:, 0:1], 0.5)
                nc.vector.memset(expshift, EXPSHIFT)
                nc.vector.memset(vw8[:, :, 512:513], 1.0)
                nc.vector.memset(vw8[:, :, 513:VW], 0.0)

                # weights lead the scalar queue (q projection is the first
                # consumer); x fp8 split across both HWDGE queues behind them
                x8 = xbp.tile([128, CT, N], fp8)
                nc.scalar.dma_start(out=wq_s, in_=w_re["q"])
                nc.scalar.dma_start(out=wkn8, in_=w_re["kn"])
                NH = 4
                HW = N // NH
                for h in range(NH):
                    for ct in range(CT):
                        eng = nc.sync if (h * CT + ct) % 2 == 0 else nc.scalar
                        eng.dma_start(
                            out=x8[:, ct, h * HW : (h + 1) * HW],
                            in_=X8[ct, :, h * HW : (h + 1) * HW],
                        )
                nc.scalar.dma_start(out=wov_s, in_=w_re["ov"])
                # token-major residual (output bias pre-added on host)
                nc.sync.dma_start(out=xrt, in_=XRT[:, :, :])

                # PE keepalive across the DMA window: junk matmuls chained
                # through tiny DVE copies (ACT's queue carries DMA issues and
                # must not gate this) so the clock is ramped when the
                # projections start
                for nwarm in range(10):
                    pw = ps_warm.tile([128, 512], f32, tag="pw")
                    nc.tensor.matmul(
                        pw, warm_sb[:, 0:128], warm_sb, start=True, stop=True
                    )
                    nc.vector.tensor_copy(out=warm_sb[:, 0:1], in_=pw[:, 0:1])

            # ---- q projection -> m8 = (Wk^T q8) fold -> S^T/exp with vw
            # projection interleaved to fill the PE while ACT runs exp ----
            with (
                tc.tile_pool(name="ps_pj", bufs=4, space="PSUM") as ps_pj,
                tc.tile_pool(name="ps_qk", bufs=2, space="PSUM") as ps_qk,
            ):
                # q projection and m8 = (Wk^T q8) fold, pipelined per
                # 512-query half: m(qh) starts as soon as its q half exists
                for jc in range(NB // 512):
                    for ot in range(CT):
                        ps = ps_pj.tile([128, 512], f32, tag="ps")
                        for cp in range(2):
                            nc.tensor.matmul(
                                ps,
                                wq_s[:, 2 * cp : 2 * cp + 2, ot * 128 : (ot + 1) * 128],
                                x8[:, 2 * cp : 2 * cp + 2, jc * 512 : (jc + 1) * 512],
                                start=(cp == 0),
                                stop=(cp == 1),
                                perf_mode=DROW,
                            )
                        # q8 = ps/WQLAM + bias_q, on DVE (ACT's queue carries
                        # the head DMA issues; exp must not queue behind this)
                        nc.vector.scalar_tensor_tensor(
                            out=q8[:, ot, jc * 512 : (jc + 1) * 512],
                            in0=ps,
                            scalar=1.0 / WQLAM,
                            in1=bias_q[:, ot, :].broadcast_to((128, 512)),
                            op0=ALU.mult,
                            op1=ALU.add,
                        )
                    qh = jc
                    for icb in range(CT):
                        ps = ps_pj.tile([128, 512], f32, tag="ps")
                        for cp in range(2):
                            nc.tensor.matmul(
                                ps,
                                wkn8[:, 2 * cp : 2 * cp + 2, icb * 128 : (icb + 1) * 128],
                                q8[:, 2 * cp : 2 * cp + 2, qh * 512 : (qh + 1) * 512],
                                start=(cp == 0),
                                stop=(cp == 1),
                                perf_mode=DROW,
                            )
                        nc.vector.tensor_scalar_mul(
                            out=m8[:, icb, qh * 512 : (qh + 1) * 512],
                            in0=ps,
                            scalar1=aqm_t[:, icb, :],
                        )

                # S^T/exp (4 key chunks) alternating with vw projection
                # (2 token pairs): PE stays full while ACT streams exp
                pbfp_cm = tc.tile_pool(name="pbfp", bufs=2)
                pbfp = pbfp_cm.__enter__()

                def qk_col(jc):
                    for kc in range(4 * jc, 4 * jc + 4):
                        ps = ps_qk.tile([128, NB], f32, tag="st")
                        for qh in range(2):
                            for cp in range(2):
                                nc.tensor.matmul(
                                    ps[:, qh * 512 : (qh + 1) * 512],
                                    x8[:, 2 * cp : 2 * cp + 2, kc * 128 : (kc + 1) * 128],
                                    m8[:, 2 * cp : 2 * cp + 2, qh * 512 : (qh + 1) * 512],
                                    start=(cp == 0),
                                    stop=(cp == 1),
                                    perf_mode=DROW,
                                )
                        if kc % 2 == 1:
                            # relieve ACT (the phase pacer): bf16 exp with
                            # the fp8 cast offloaded to DVE
                            pbf = pbfp.tile([128, NB], bf16, tag="pbf")
                            nc.scalar.activation(
                                out=pbf,
                                in_=ps,
                                func=ACT.Exp,
                                bias=expshift,
                                scale=1.0 / M8LAM,
                            )
                            nc.vector.tensor_copy(out=p8[:, kc, :], in_=pbf)
                        else:
                            nc.scalar.activation(
                                out=p8[:, kc, :],
                                in_=ps,
                                func=ACT.Exp,
                                bias=expshift,
                                scale=1.0 / M8LAM,
                            )

                def vw_tile(tb):
                    ps = ps_pj.tile([128, 512], f32, tag="ps")
                    for cp in range(2):
                        nc.tensor.matmul(
                            ps,
                            x8[:, 2 * cp : 2 * cp + 2, tb * 128 : (tb + 1) * 128],
                            wov_s[:, 2 * cp : 2 * cp + 2, :],
                            start=(cp == 0),
                            stop=(cp == 1),
                            perf_mode=DROW,
                        )
                    # all vw evacuations on DVE: ACT is saturated by exp
                    nc.vector.tensor_scalar_mul(
                        out=vw8[:, tb, 0:512],
                        in0=ps,
                        scalar1=1.0 / WVLAM,
                    )

                for jc in range(N // 512):
                    qk_col(jc)
                    for tb in range(4 * jc, 4 * jc + 4):
                        vw_tile(tb)
                pbfp_cm.__exit__(None, None, None)

            # ---- AV in fp8 DoubleRow; ones-column gives den at pav[:,766] ----
            with (
                tc.tile_pool(name="loopp", bufs=3) as loopp,
                tc.tile_pool(name="ps_av", bufs=2, space="PSUM") as ps_av,
            ):
                for qs in range(NB // 128):
                    pav = ps_av.tile([128, 1024], f32, tag="pav")
                    for j in range(N // 256):
                        stat = p8[:, 2 * j : 2 * j + 2, qs * 128 : (qs + 1) * 128]
                        nc.tensor.matmul(
                            pav[:, 0:AVS],
                            stat,
                            vw8[:, 2 * j : 2 * j + 2, 0:AVS],
                            start=(j == 0),
                            stop=(j == N // 256 - 1),
                            perf_mode=DROW,
                        )
                        nc.tensor.matmul(
                            pav[:, 512 : 512 + (VW - AVS)],
                            stat,
                            vw8[:, 2 * j : 2 * j + 2, AVS:VW],
                            start=(j == 0),
                            stop=(j == N // 256 - 1),
                            perf_mode=DROW,
                        )
                    rden = loopp.tile([128, 1], f32, tag="rden")
                    nc.vector.reciprocal(
                        out=rden, in_=pav[:, 512 + 512 - AVS : 512 + 512 - AVS + 1]
                    )
                    outf = loopp.tile([128, C], f32, tag="outf")
                    nc.vector.scalar_tensor_tensor(
                        out=outf[:, 0:AVS],
                        in0=pav[:, 0:AVS],
                        scalar=rden,
                        in1=xrt[:, qs, 0:AVS],
                        op0=ALU.mult,
                        op1=ALU.add,
                    )
                    nc.vector.scalar_tensor_tensor(
                        out=outf[:, AVS:C],
                        in0=pav[:, 512 : 512 + C - AVS],
                        scalar=rden,
                        in1=xrt[:, qs, AVS:C],
                        op0=ALU.mult,
                        op1=ALU.add,
                    )
                    eng = nc.sync if qs % 2 == 0 else nc.scalar
                    eng.dma_start(out=OUT[qs], in_=outf)

    if split_waits:
        _split_sync_waits(nc)
    return nc


def _prep_in_maps(x, gn_gamma, gn_beta, wq, bq, wk, bk, wv, bv, wo, bo):
    import ml_dtypes

    f = np.float32
    f8 = ml_dtypes.float8_e4m3  # matches mybir.dt.float8e4's layout

    xr = np.asarray(x, f).reshape(2, C, N)
    wqf = np.asarray(wq, f)
    wkf = np.asarray(wk, f)
    wov = np.asarray(wo, f) @ np.asarray(wv, f)
    bias_o0 = np.asarray(bo, f) + np.asarray(wo, f) @ np.asarray(bv, f)
    gam = np.asarray(gn_gamma, f)
    bet = np.asarray(gn_beta, f)
    bqf = np.asarray(bq, f)

    # GroupNorm folded per channel (exact f32 moments, per batch):
    # hn = a*x + d
    xg = xr.reshape(2, G, C // G * N)
    mu = xg.mean(axis=2)                      # (2, G)
    var = xg.var(axis=2)                      # (2, G)

    # per-batch per-channel a, d
    a_bc = np.empty((2, C), f)
    d_bc = np.empty((2, C), f)
    for b in range(2):
        ac = gam / np.sqrt(var[b].repeat(C // G) + EPS)
        a_bc[b] = ac
        d_bc[b] = bet - ac * mu[b].repeat(C // G)

    # raw (untransposed) Wk, prescaled into fp8's sweet spot
    wk_n8 = np.ascontiguousarray((wkf * WKLAM).astype(f8))

    def vec(v):
        return np.ascontiguousarray(
            np.asarray(v, f).reshape(CT, 128).transpose(1, 0)
        )

    cidx = np.arange(C)

    in_maps = []
    for core in range(NCORES):
        b, r = divmod(core, 4)
        a = a_bc[b]
        d = d_bc[b]
        # folded fp8 weights (transposed layout [ic, oc]); prescales are
        # undone by the on-device evacuation scales
        wq_s8 = np.ascontiguousarray(
            (wqf.T * (a * QKSCALE * WQLAM)[:, None]).astype(f8)
        )
        wov_s8 = np.ascontiguousarray((wov.T * (a * WVLAM)[:, None]).astype(f8))
        # q bias after GN fold: s*(Wq d + bq)
        bias_qv = QKSCALE * (wqf @ d + bqf)
        # m8 evacuation fold
        aqm = a * (QKSCALE * M8LAM / WKLAM)
        smalls = np.zeros((128, CT, 2), f)
        smalls[:, :, 0] = vec(bias_qv)
        smalls[:, :, 1] = vec(aqm)

        xroll = np.ascontiguousarray(np.roll(xr[b], -r * NB, axis=1).reshape(CT, 128, N))
        xres_t = np.ascontiguousarray(
            (xroll.reshape(C, N)[:, :NB].T + bias_o0[None, :])
            .reshape(NB // 128, 128, C)
            .transpose(1, 0, 2)
        )
        in_maps.append(
            {
                "x_f8": xroll.astype(f8),
                "xres_t": xres_t,
                "wq_s8": wq_s8,
                "wk_n8": wk_n8,
                "wov_s8": wov_s8,
                "smalls": smalls,
            }
        )
    return in_maps


def _assemble(results):
    out = np.empty((2, C, N), np.float32)
    for core in range(NCORES):
        b, r = divmod(core, 4)
        out[b][:, r * NB : (r + 1) * NB] = (
            np.asarray(results[core]["out"]).reshape(NB, C).T
        )
    return out.reshape(2, C, 64, 64)


def _run(in_maps, trace=False, trace_kwargs=None):
    from concourse.bass_utils import run_bass_kernel_spmd

    if "nc" not in _cache:
        _cache["nc"] = build()
    kw = {}
    if trace:
        kw = {"trace": True, "trace_kwargs": trace_kwargs or {}}
    return run_bass_kernel_spmd(
        _cache["nc"], in_maps, core_ids=list(range(NCORES)), **kw
    )


def kernel(x, gn_gamma, gn_beta, wq, bq, wk, bk, wv, bv, wo, bo):
    in_maps = _prep_in_maps(x, gn_gamma, gn_beta, wq, bq, wk, bk, wv, bv, wo, bo)
    res = _run(in_maps, trace=False)
    return _assemble(res.results)


# revision 34
# speedup vs baseline: 1.2149x; 1.2149x over previous
"""AttnBlock (GroupNorm -> qkv 1x1 -> softmax attention -> proj -> residual)
for x (2, 512, 64, 64) on 8 Trainium2 NeuronCores.

Sharding: core i handles batch i//4 and query-token block i%4 (1024 of 4096
spatial tokens). k/v are computed per-core over all 4096 tokens (no
collectives). Inputs are token-rolled per core so every core runs the same
SPMD graph with its own query block at token offset 0.

Following the original baseline's host-prep style (wo@wv product, bias and
residual folds, dtype casts), the cheap per-channel algebra is folded on the
host: GroupNorm reduces to hn = a*x + d (a,d per channel from exact f32
moments) and is absorbed into fp8 copies of the projection weights, with
per-tensor power-of-two prescales so fp8 sees a healthy range (undone by
the evacuation/activation scales on device). The ~86 GFLOP of projections
and attention all run on device.

The k projection is eliminated algebraically: S^T = K^T Q = x8^T (Wk_s Q),
so a small m8 = Wk^T q8 replaces the full 4096-token K — the S^T matmuls
use raw x8 chunks as stationary. exp(s/4 - 3) streams PSUM->SBUF into fp8
P^T (no max pass; the shift keeps fp8 in range and cancels in the softmax
ratio). The vw projection interleaves with the S^T/exp stream to fill the
PE while exp runs on ACT. A@V is all-fp8 DoubleRow with P^T chunks
stationary; the softmax denominator folds into AV via a ones-column in vw8
with each AV step bank-split (cols 0:258 -> bank0, 258:516 -> bank1), so
den[q] lands per-partition at pav[:,766] for free.
"""

import numpy as np

C = 512          # channels
N = 4096         # spatial tokens (64*64)
NB = 1024        # query tokens per core
G = 32           # groups
CT = 4           # channel tiles of 128
EPS = 1e-6
QKSCALE = float(C) ** -0.25  # split between q and the k-side fold
NCORES = 8
VW = 516         # vw8 inner: 512 ch + ones col + 3 zero pad
AVS = 258        # AV bank split point
EXPSHIFT = -3.0  # exp(s-3): keeps fp8 P below e4m3's 240 max (scores ~ +-7.5)
WQLAM = 16.0     # host prescale on folded Wq for fp8 range
WKLAM = 8.0      # host prescale on raw Wk for fp8 range
WVLAM = 8.0      # host prescale on folded WoWv for fp8 range
M8LAM = 4.0      # extra prescale kept inside m8, undone by exp input scale

_cache = {}


def _split_sync_waits(nc, maxw=1):
    """This walrus build encodes at most ~1 sync wait per instruction
    descriptor. Move excess sem waits onto same-engine nops inserted just
    before the instruction (in-order sequencers make this equivalent)."""
    from concourse import mybir

    n = 0
    for fn in nc.m.functions:
        for b in fn.blocks:
            out = []
            for ins in b.instructions:
                si = getattr(ins, "sync_info", None)
                if si is not None and si.on_wait and len(si.on_wait) > maxw:
                    waits = list(si.on_wait)
                    extra, keep = waits[:-maxw], waits[-maxw:]
                    for j in range(0, len(extra), maxw):
                        nop = mybir.InstNoOp(name=f"I-wsp{n}", ins=[], outs=[])
                        n += 1
                        nop.engine = ins.engine
                        nop.sync_info = mybir.SyncInfo(
                            on_wait=extra[j : j + maxw], on_update=[]
                        )
                        out.append(nop)
                    ins.sync_info = mybir.SyncInfo(
                        on_wait=keep, on_update=list(si.on_update)
                    )
                out.append(ins)
            b.instructions = out


def build(split_waits=True):
    import concourse.bass as bass
    import concourse.tile as tile
    from concourse import mybir

    f32 = mybir.dt.float32
    bf16 = mybir.dt.bfloat16
    fp8 = mybir.dt.float8e4
    ALU = mybir.AluOpType
    ACT = mybir.ActivationFunctionType
    DROW = mybir.MatmulPerfMode.DoubleRow

    nc = bass.Bass()
    X8 = nc.declare_dram_parameter("x_f8", [CT, 128, N], fp8, isOutput=False)
    WQ8 = nc.declare_dram_parameter("wq_s8", [C, C], fp8, isOutput=False)
    WKN = nc.declare_dram_parameter("wk_n8", [C, C], fp8, isOutput=False)
    WOV8 = nc.declare_dram_parameter("wov_s8", [C, C], fp8, isOutput=False)
    SML = nc.declare_dram_parameter("smalls", [128, CT, 2], f32, isOutput=False)
    XRT = nc.declare_dram_parameter("xres_t", [128, NB // 128, C], f32, isOutput=False)
    OUT = nc.declare_dram_parameter("out", [NB // 128, 128, C], f32, isOutput=True)

    w_re = {
        "q": WQ8.rearrange("(a p) o -> p a o", p=128),
        "kn": WKN.rearrange("(a p) o -> p a o", p=128),
        "ov": WOV8.rearrange("(a p) o -> p a o", p=128),
    }

    with tile.TileContext(nc) as tc:
        with (
            tc.tile_pool(name="persist", bufs=1) as persist,
        ):
            # persistent tensors
            xrt = persist.tile([128, NB // 128, C], f32)
            wq_s = persist.tile([128, CT, C], fp8)
            wkn8 = persist.tile([128, CT, C], fp8)
            wov_s = persist.tile([128, CT, C], fp8)
            q8 = persist.tile([128, CT, NB], fp8)
            m8 = persist.tile([128, CT, NB], fp8)
            vw8 = persist.tile([128, N // 128, VW], fp8)
            p8 = persist.tile([128, N // 128, NB], fp8)
            expshift = persist.tile([128, 1], f32)

            with (
                tc.tile_pool(name="xbp", bufs=1) as xbp,
                tc.tile_pool(name="statp", bufs=2) as statp,
                tc.tile_pool(name="ps_warm", bufs=1, space="PSUM") as ps_warm,
            ):
                # folded per-channel scalars (per core/batch, from host):
                # [:, :, 0] = q bias (after GN fold), [:, :, 1] = m8 fold
                sml_t = statp.tile([128, CT, 2], f32, tag="sml")
                nc.sync.dma_start(out=sml_t, in_=SML[:, :, :])
                bias_q = sml_t[:, :, 0:1]
                aqm_t = sml_t[:, :, 1:2]
                nc.vector.memset(expshift, EXPSHIFT)
                nc.vector.memset(vw8[:, :, 512:513], 1.0)
                nc.vector.memset(vw8[:, :, 513:VW], 0.0)

                # weights lead the scalar queue (q projection is the first
                # consumer); x fp8 split across both HWDGE queues behind them
                x8 = xbp.tile([128, CT, N], fp8)
                nc.scalar.dma_start(out=wq_s, in_=w_re["q"])
                nc.scalar.dma_start(out=wkn8, in_=w_re["kn"])
                NH = 4
                HW = N // NH
                for h in range(NH):
                    for ct in range(CT):
                        eng = nc.sync if (h * CT + ct) % 2 == 0 else nc.scalar
                        eng.dma_start(
                            out=x8[:, ct, h * HW : (h + 1) * HW],
                            in_=X8[ct, :, h * HW : (h + 1) * HW],
                        )
                nc.scalar.dma_start(out=wov_s, in_=w_re["ov"])
                # token-major residual (output bias pre-added on host)
                nc.sync.dma_start(out=xrt, in_=XRT[:, :, :])

                # PE keepalive across the DMA window: junk matmuls chained
                # through tiny DVE copies (ACT's queue carries DMA issues and
                # must not gate this) so the clock is ramped when the
                # projections start
                warm_sb = statp.tile([128, 512], bf16, tag="warm_sb")
                nc.vector.memset(warm_sb[:, 0:1], 0.5)
                for nwarm in range(6):
                    pw = ps_warm.tile([128, 512], f32, tag="pw")
                    nc.tensor.matmul(
                        pw, warm_sb[:, 0:128], warm_sb, start=True, stop=True
                    )
                    nc.vector.tensor_copy(out=warm_sb[:, 0:1], in_=pw[:, 0:1])

            # ---- q projection -> m8 = (Wk^T q8) fold -> S^T/exp with vw
            # projection interleaved to fill the PE while ACT runs exp ----
            with (
                tc.tile_pool(name="ps_pj", bufs=4, space="PSUM") as ps_pj,
                tc.tile_pool(name="ps_qk", bufs=2, space="PSUM") as ps_qk,
            ):
                # q projection and m8 = (Wk^T q8) fold, pipelined per
                # 512-query half: m(qh) starts as soon as its q half exists
                for jc in range(NB // 512):
                    for ot in range(CT):
                        ps = ps_pj.tile([128, 512], f32, tag="ps")
                        for cp in range(2):
                            nc.tensor.matmul(
                                ps,
                                wq_s[:, 2 * cp : 2 * cp + 2, ot * 128 : (ot + 1) * 128],
                                x8[:, 2 * cp : 2 * cp + 2, jc * 512 : (jc + 1) * 512],
                                start=(cp == 0),
                                stop=(cp == 1),
                                perf_mode=DROW,
                            )
                        # q8 = ps/WQLAM + bias_q, on DVE (ACT's queue carries
                        # the head DMA issues; exp must not queue behind this)
                        nc.vector.scalar_tensor_tensor(
                            out=q8[:, ot, jc * 512 : (jc + 1) * 512],
                            in0=ps,
                            scalar=1.0 / WQLAM,
                            in1=bias_q[:, ot, :].broadcast_to((128, 512)),
                            op0=ALU.mult,
                            op1=ALU.add,
                        )
                    qh = jc
                    for icb in range(CT):
                        ps = ps_pj.tile([128, 512], f32, tag="ps")
                        for cp in range(2):
                            nc.tensor.matmul(
                                ps,
                                wkn8[:, 2 * cp : 2 * cp + 2, icb * 128 : (icb + 1) * 128],
                                q8[:, 2 * cp : 2 * cp + 2, qh * 512 : (qh + 1) * 512],
                                start=(cp == 0),
                                stop=(cp == 1),
                                perf_mode=DROW,
                            )
                        nc.vector.tensor_scalar_mul(
                            out=m8[:, icb, qh * 512 : (qh + 1) * 512],
                            in0=ps,
                            scalar1=aqm_t[:, icb, :],
                        )

                # S^T/exp (4 key chunks) alternating with vw projection
                # (2 token pairs): PE stays full while ACT streams exp
                pbfp_cm = tc.tile_pool(name="pbfp", bufs=2)
                pbfp = pbfp_cm.__enter__()

                def qk_col(jc):
                    for kc in range(4 * jc, 4 * jc + 4):
                        ps = ps_qk.tile([128, NB], f32, tag="st")
                        for qh in range(2):
                            for cp in range(2):
                                nc.tensor.matmul(
                                    ps[:, qh * 512 : (qh + 1) * 512],
                                    x8[:, 2 * cp : 2 * cp + 2, kc * 128 : (kc + 1) * 128],
                                    m8[:, 2 * cp : 2 * cp + 2, qh * 512 : (qh + 1) * 512],
                                    start=(cp == 0),
                                    stop=(cp == 1),
                                    perf_mode=DROW,
                                )
                        if kc % 4 == 1:
                            # relieve ACT (the phase pacer): bf16 exp with
                            # the fp8 cast offloaded to DVE
                            pbf = pbfp.tile([128, NB], bf16, tag="pbf")
                            nc.scalar.activation(
                                out=pbf,
                                in_=ps,
                                func=ACT.Exp,
                                bias=expshift,
                                scale=1.0 / M8LAM,
                            )
                            nc.vector.tensor_copy(out=p8[:, kc, :], in_=pbf)
                        else:
                            nc.scalar.activation(
                                out=p8[:, kc, :],
                                in_=ps,
                                func=ACT.Exp,
                                bias=expshift,
                                scale=1.0 / M8LAM,
                            )

                def vw_tile(tb):
                    ps = ps_pj.tile([128, 512], f32, tag="ps")
                    for cp in range(2):
                        nc.tensor.matmul(
                            ps,
                            x8[:, 2 * cp : 2 * cp + 2, tb * 128 : (tb + 1) * 128],
                            wov_s[:, 2 * cp : 2 * cp + 2, :],
                            start=(cp == 0),
                            stop=(cp == 1),
                            perf_mode=DROW,
                        )
                    # all vw evacuations on DVE: ACT is saturated by exp
                    nc.vector.tensor_scalar_mul(
                        out=vw8[:, tb, 0:512],
                        in0=ps,
                        scalar1=1.0 / WVLAM,
                    )

                for jc in range(N // 512):
                    qk_col(jc)
                    for tb in range(4 * jc, 4 * jc + 4):
                        vw_tile(tb)
                pbfp_cm.__exit__(None, None, None)

            # ---- AV in fp8 DoubleRow; ones-column gives den at pav[:,766] ----
            with (
                tc.tile_pool(name="loopp", bufs=3) as loopp,
                tc.tile_pool(name="ps_av", bufs=2, space="PSUM") as ps_av,
            ):
                for qs in range(NB // 128):
                    pav = ps_av.tile([128, 1024], f32, tag="pav")
                    for j in range(N // 256):
                        stat = p8[:, 2 * j : 2 * j + 2, qs * 128 : (qs + 1) * 128]
                        nc.tensor.matmul(
                            pav[:, 0:AVS],
                            stat,
                            vw8[:, 2 * j : 2 * j + 2, 0:AVS],
                            start=(j == 0),
                            stop=(j == N // 256 - 1),
                            perf_mode=DROW,
                        )
                        nc.tensor.matmul(
                            pav[:, 512 : 512 + (VW - AVS)],
                            stat,
                            vw8[:, 2 * j : 2 * j + 2, AVS:VW],
                            start=(j == 0),
                            stop=(j == N // 256 - 1),
                            perf_mode=DROW,
                        )
                    rden = loopp.tile([128, 1], f32, tag="rden")
                    nc.vector.reciprocal(
                        out=rden, in_=pav[:, 512 + 512 - AVS : 512 + 512 - AVS + 1]
                    )
                    outf = loopp.tile([128, C], f32, tag="outf")
                    nc.vector.scalar_tensor_tensor(
                        out=outf[:, 0:AVS],
                        in0=pav[:, 0:AVS],
                        scalar=rden,
                        in1=xrt[:, qs, 0:AVS],
                        op0=ALU.mult,
                        op1=ALU.add,
                    )
                    nc.vector.scalar_tensor_tensor(
                        out=outf[:, AVS:C],
                        in0=pav[:, 512 : 512 + C - AVS],
                        scalar=rden,
                        in1=xrt[:, qs, AVS:C],
                        op0=ALU.mult,
                        op1=ALU.add,
                    )
                    eng = nc.sync if qs % 2 == 0 else nc.scalar
                    eng.dma_start(out=OUT[qs], in_=outf)

    if split_waits:
        _split_sync_waits(nc)
    return nc


def _prep_in_maps(x, gn_gamma, gn_beta, wq, bq, wk, bk, wv, bv, wo, bo):
    import ml_dtypes

    f = np.float32
    f8 = ml_dtypes.float8_e4m3  # matches mybir.dt.float8e4's layout

    xr = np.asarray(x, f).reshape(2, C, N)
    wqf = np.asarray(wq, f)
    wkf = np.asarray(wk, f)
    wov = np.asarray(wo, f) @ np.asarray(wv, f)
    bias_o0 = np.asarray(bo, f) + np.asarray(wo, f) @ np.asarray(bv, f)
    gam = np.asarray(gn_gamma, f)
    bet = np.asarray(gn_beta, f)
    bqf = np.asarray(bq, f)

    # GroupNorm folded per channel (exact f32 moments, per batch):
    # hn = a*x + d
    xg = xr.reshape(2, G, C // G * N)
    mu = xg.mean(axis=2)                      # (2, G)
    var = xg.var(axis=2)                      # (2, G)

    # per-batch per-channel a, d
    a_bc = np.empty((2, C), f)
    d_bc = np.empty((2, C), f)
    for b in range(2):
        ac = gam / np.sqrt(var[b].repeat(C // G) + EPS)
        a_bc[b] = ac
        d_bc[b] = bet - ac * mu[b].repeat(C // G)

    # raw (untransposed) Wk, prescaled into fp8's sweet spot
    wk_n8 = np.ascontiguousarray((wkf * WKLAM).astype(f8))

    def vec(v):
        return np.ascontiguousarray(
            np.asarray(v, f).reshape(CT, 128).transpose(1, 0)
        )

    cidx = np.arange(C)

    in_maps = []
    for core in range(NCORES):
        b, r = divmod(core, 4)
        a = a_bc[b]
        d = d_bc[b]
        # folded fp8 weights (transposed layout [ic, oc]); prescales are
        # undone by the on-device evacuation scales
        wq_s8 = np.ascontiguousarray(
            (wqf.T * (a * QKSCALE * WQLAM)[:, None]).astype(f8)
        )
        wov_s8 = np.ascontiguousarray((wov.T * (a * WVLAM)[:, None]).astype(f8))
        # q bias after GN fold: s*(Wq d + bq)
        bias_qv = QKSCALE * (wqf @ d + bqf)
        # m8 evacuation fold
        aqm = a * (QKSCALE * M8LAM / WKLAM)
        smalls = np.zeros((128, CT, 2), f)
        smalls[:, :, 0] = vec(bias_qv)
        smalls[:, :, 1] = vec(aqm)

        xroll = np.ascontiguousarray(np.roll(xr[b], -r * NB, axis=1).reshape(CT, 128, N))
        xres_t = np.ascontiguousarray(
            (xroll.reshape(C, N)[:, :NB].T + bias_o0[None, :])
            .reshape(NB // 128, 128, C)
            .transpose(1, 0, 2)
        )
        in_maps.append(
            {
                "x_f8": xroll.astype(f8),
                "xres_t": xres_t,
                "wq_s8": wq_s8,
                "wk_n8": wk_n8,
                "wov_s8": wov_s8,
                "smalls": smalls,
            }
        )
    return in_maps


def _assemble(results):
    out = np.empty((2, C, N), np.float32)
    for core in range(NCORES):
        b, r = divmod(core, 4)
        out[b][:, r * NB : (r + 1) * NB] = (
            np.asarray(results[core]["out"]).reshape(NB, C).T
        )
    return out.reshape(2, C, 64, 64)


def _run(in_maps, trace=False, trace_kwargs=None):
    from concourse.bass_utils import run_bass_kernel_spmd

    if "nc" not in _cache:
        _cache["nc"] = build()
    kw = {}
    if trace:
        kw = {"trace": True, "trace_kwargs": trace_kwargs or {}}
    return run_bass_kernel_spmd(
        _cache["nc"], in_maps, core_ids=list(range(NCORES)), **kw
    )


def kernel(x, gn_gamma, gn_beta, wq, bq, wk, bk, wv, bv, wo, bo):
    in_maps = _prep_in_maps(x, gn_gamma, gn_beta, wq, bq, wk, bk, wv, bv, wo, bo)
    res = _run(in_maps, trace=False)
    return _assemble(res.results)
